# revision 1
# baseline (speedup 1.0000x reference)
"""Trainium2 Bass kernel for nn_CLGNN_Model (3-layer GCN + MLP head + log_softmax).

Sharding: nodes are partitioned across 8 NeuronCores (12500 each).  Per GCN
layer, each core computes z = h @ W for its own nodes, scales rows by
dinv = rsqrt(deg), casts to bf16 and AllGathers the resulting "message table"
[100352, 256].  Edges are assigned to the core owning their destination;
the aggregation  acc[dst] = sum_{e->dst} g[src_e]  is computed with
dma_gather (int16-indexed row gather from the table, chunked into 4 source
ranges of 25088 rows to fit int16) followed by 0/1-indicator matmuls on the
TensorEngine that segment-sum 128 gathered edge rows at a time into a PSUM
accumulator per 128-destination window.  Self-loops (weight 2.0) are encoded
as two duplicate edges.  The instruction stream is identical across cores
(group counts are maxed over cores, short cells padded with sentinel edges)
so one SPMD program serves all 8 cores; only the data arrays differ.
"""
import sys
import os
import hashlib
from dataclasses import dataclass

sys.path.insert(0, "/opt/trn_rl_repo")

import numpy as np
import ml_dtypes

BF16 = ml_dtypes.bfloat16

# ----------------------------------------------------------------------------
# configuration
# ----------------------------------------------------------------------------


@dataclass(frozen=True)
class Cfg:
    N: int = 100000           # total nodes
    NFEAT: int = 512
    NLABEL: int = 64
    NHID: int = 256
    NCORES: int = 8
    P: int = 128
    SW: int = 3               # windows per superwindow
    IND_B: int = 8            # indicator groups per DVE op

    @property
    def NOWN(self):           # nodes per core
        return self.N // self.NCORES

    @property
    def NW(self):             # 128-windows per core
        return (self.NOWN + self.P - 1) // self.P

    @property
    def NOWN_PAD(self):
        return self.NW * self.P

    @property
    def TBL_ROWS(self):
        return self.NCORES * self.NOWN_PAD

    @property
    def CHUNK(self):          # table rows per int16-addressable chunk
        return 2 * self.NOWN_PAD

    @property
    def NCHUNK(self):
        return 4

    @property
    def NSW(self):
        return (self.NW + self.SW - 1) // self.SW

    @property
    def DIN(self):            # GCN layer-0 input dim
        return self.NFEAT + self.NLABEL

    @property
    def KIN0(self):           # 128-chunks of DIN (padded)
        return (self.DIN + self.P - 1) // self.P


FULL = Cfg()
PAD_SENTINEL = 200.0

# ----------------------------------------------------------------------------
# host-side preprocessing
# ----------------------------------------------------------------------------


def _build_feats(cfg, x, y, idx_labeled):
    n = x.shape[0]
    idx = np.full((n,), cfg.NLABEL + 2, np.int64)
    idx[idx_labeled] = y[idx_labeled]
    feats = np.zeros((n, cfg.NLABEL), np.float32)
    lab = idx < cfg.NLABEL
    feats[np.nonzero(lab)[0], idx[lab]] = 1.0
    return np.concatenate([x, feats], axis=1)


def _build_schedule(cfg, adj):
    """Device-independent schedule + per-device index/dst arrays."""
    P = cfg.P
    src = adj[0].astype(np.int64)
    dst = adj[1].astype(np.int64)

    indeg = np.bincount(dst, minlength=cfg.N).astype(np.float32)
    deg_full = indeg + 2.0

    # cells in schedule order: sw asc -> chunk asc -> window asc
    cells = []            # (sw, c, w)
    cell_id = {}
    for s in range(cfg.NSW):
        ws = range(s * cfg.SW, min((s + 1) * cfg.SW, cfg.NW))
        for c in range(cfg.NCHUNK):
            for w in ws:
                cell_id[(c, w)] = len(cells)
                cells.append((s, c, w))
    ncells = len(cells)

    # per-device edge -> cell assignment
    dev_edges = []        # (cell, src_local, dst_rel) arrays per device
    counts = np.zeros((cfg.NCORES, ncells), np.int64)
    for d in range(cfg.NCORES):
        mask = (dst // cfg.NOWN) == d
        es = src[mask]
        ed = dst[mask]
        dl = ed - d * cfg.NOWN
        w = dl // P
        dst_rel = (dl % P).astype(np.float32)
        trow = (es // cfg.NOWN) * cfg.NOWN_PAD + (es % cfg.NOWN)
        c = trow // cfg.CHUNK
        src_local = (trow % cfg.CHUNK).astype(np.int64)
        # vectorized cell id (cells iterate sw -> c -> w)
        sw_of = w // cfg.SW
        # offset of sw block
        sw_sizes = [min((s + 1) * cfg.SW, cfg.NW) - s * cfg.SW
                    for s in range(cfg.NSW)]
        sw_off = np.cumsum([0] + [sz * cfg.NCHUNK for sz in sw_sizes])[:-1]
        sw_sz = np.array(sw_sizes)[sw_of]
        w_in_sw = w - sw_of * cfg.SW
        cidv = sw_off[sw_of] + c * sw_sz + w_in_sw
        np.add.at(counts[d], cidv, 1)
        dev_edges.append((cidv, src_local, dst_rel))

    G = (counts.max(axis=0) + P - 1) // P   # groups per cell (0 if empty)
    cap = G * P
    cell_off = np.concatenate([[0], np.cumsum(cap)])       # edge offsets
    G_off = np.concatenate([[0], np.cumsum(G)])            # group offsets
    G_total = int(G.sum())
    total = int(cap.sum())

    # per-group metadata (window, start, stop) in schedule order
    groups = np.empty((G_total, 3), np.int64)
    first_seen = {}
    last_group_of_w = {}
    for ci, (s, c, w) in enumerate(cells):
        for k in range(G[ci]):
            gi = G_off[ci] + k
            st = w not in first_seen
            first_seen[w] = True
            groups[gi] = (w, 1 if st else 0, 0)
            last_group_of_w[w] = gi
    for w, gi in last_group_of_w.items():
        groups[gi][2] = 1

    # per-sw call structure (chunk, n_groups, group offset)
    sw_calls = []
    sw_g_off = []
    sw_g_cnt = []
    ci = 0
    for s in range(cfg.NSW):
        ws = range(s * cfg.SW, min((s + 1) * cfg.SW, cfg.NW))
        calls = []
        g0 = G_off[ci]
        for c in range(cfg.NCHUNK):
            ng = 0
            goff = G_off[ci]
            for _ in ws:
                ng += int(G[ci])
                ci += 1
            calls.append((c, ng, int(goff - g0)))
        sw_calls.append(calls)
        sw_g_off.append(int(g0))
        sw_g_cnt.append(int(G_off[ci] - g0))
    G_SW_MAX = max(sw_g_cnt)

    # per-device data arrays
    dev_idx = []
    dev_dstv = []
    dev_deg = []
    for d in range(cfg.NCORES):
        cidv, src_local, dst_rel = dev_edges[d]
        # ascending src within each cell -> better HBM locality for gathers
        order = np.lexsort((src_local, cidv))
        cid_s = cidv[order]
        starts = np.searchsorted(cid_s, np.arange(ncells))
        within = np.arange(len(cid_s)) - starts[cid_s]
        pos = cell_off[cid_s] + within
        idx_flat = np.zeros(total, np.int64)
        dr_flat = np.full(total, PAD_SENTINEL, np.float32)
        idx_flat[pos] = src_local[order]
        dr_flat[pos] = dst_rel[order]
        # wrapped int16 layout [128, total//16]
        a = idx_flat.reshape(total // 16, 16).T.astype(np.int16)
        dev_idx.append(np.ascontiguousarray(np.tile(a, (8, 1))))
        dev_dstv.append(np.ascontiguousarray(
            dr_flat.reshape(G_total, P).T.astype(BF16)))
        dg = np.full((cfg.NOWN_PAD,), 1.0, np.float32)
        dg[:cfg.NOWN] = deg_full[d * cfg.NOWN:(d + 1) * cfg.NOWN]
        dev_deg.append(np.ascontiguousarray(
            dg.reshape(cfg.NW, P).T))          # [128, NW]

    sched = dict(
        groups=groups, sw_calls=sw_calls, sw_g_off=sw_g_off,
        sw_g_cnt=sw_g_cnt, G_SW_MAX=int(G_SW_MAX), G_total=G_total,
        S_total=G_total * 8,
    )
    return sched, dev_idx, dev_dstv, dev_deg


def _pack_h0(cfg, h0_dev):
    """[NOWN_PAD, DIN] f32 -> [NW, 128, KIN0*128] bf16 lhsT-packed."""
    dpad = cfg.KIN0 * cfg.P
    h = np.zeros((cfg.NOWN_PAD, dpad), np.float32)
    h[:, :cfg.DIN] = h0_dev
    # [t*128+nc, kc*128+p] -> out[t, p, kc*128+nc]
    v = h.reshape(cfg.NW, cfg.P, cfg.KIN0, cfg.P)      # t, nc, kc, p
    return np.ascontiguousarray(v.transpose(0, 3, 2, 1)
                                .reshape(cfg.NW, cfg.P, cfg.KIN0 * cfg.P)
                                .astype(BF16))


def _pack_w(W, kin_chunks, p=128):
    """[K, O] -> [kin_chunks, 128, O] bf16 (zero-padded)."""
    K, O = W.shape
    Wp = np.zeros((kin_chunks * p, O), np.float32)
    Wp[:K] = W
    return np.ascontiguousarray(
        Wp.reshape(kin_chunks, p, O).astype(BF16))


def _bcast(b, p=128):
    return np.ascontiguousarray(np.broadcast_to(
        b.astype(np.float32)[None, :], (p, len(b))).copy())


# ----------------------------------------------------------------------------
# Bass program
# ----------------------------------------------------------------------------


DBG_LAYERS = 3        # how many GCN layers to emit
DBG_HEAD = True       # emit MLP head; if False, dump hTa/hTb to out instead
DBG_PHASE_A = True    # emit phase A; if False, dump table sample
DBG_PA_IND = True     # emit indicator builds
DBG_PA_MM = True      # emit segment-sum matmuls (needs IND)
DBG_PA_POST = True    # emit postlude (needs MM)


def _build_nc(cfg, sched):
    from concourse import bass, mybir, tile, bacc
    from concourse.masks import make_identity
    from contextlib import ExitStack

    fp32 = mybir.dt.float32
    bf16 = mybir.dt.bfloat16
    i16 = mybir.dt.int16
    P = cfg.P
    NH = cfg.NHID
    NW = cfg.NW
    KIN0 = cfg.KIN0
    G_SW_MAX = sched["G_SW_MAX"]
    groups = sched["groups"]
    IND_B = cfg.IND_B

    nc = bacc.Bacc("TRN2", debug=False, num_swdge_queues=4)

    hT0_d = nc.dram_tensor("hT0", [NW, P, KIN0 * P], bf16, kind="ExternalInput")
    idx_d = nc.dram_tensor("idx", [P, sched["S_total"]], i16, kind="ExternalInput")
    dstv_d = nc.dram_tensor("dstv", [P, sched["G_total"]], bf16, kind="ExternalInput")
    deg_d = nc.dram_tensor("deg", [P, NW], fp32, kind="ExternalInput")
    w0_d = nc.dram_tensor("w0", [KIN0, P, NH], bf16, kind="ExternalInput")
    w12_d = nc.dram_tensor("w12", [2, 2, P, NH], bf16, kind="ExternalInput")
    wm0_d = nc.dram_tensor("wm0", [2, P, 2 * NH], bf16, kind="ExternalInput")
    wm1_d = nc.dram_tensor("wm1", [4, P, 64], bf16, kind="ExternalInput")
    b012_d = nc.dram_tensor("b012", [3, P, NH], fp32, kind="ExternalInput")
    bm0_d = nc.dram_tensor("bm0", [P, 2 * NH], fp32, kind="ExternalInput")
    bm1_d = nc.dram_tensor("bm1", [P, 64], fp32, kind="ExternalInput")
    iota_d = nc.dram_tensor("iota", [P, IND_B * P], bf16, kind="ExternalInput")
    out_d = nc.dram_tensor("out", [NW, P, 64], fp32, kind="ExternalOutput")

    with tile.TileContext(nc) as tc, ExitStack() as ctx:
        const = ctx.enter_context(tc.tile_pool(name="const", bufs=1))
        ht = ctx.enter_context(tc.tile_pool(name="ht", bufs=1))
        work = ctx.enter_context(tc.tile_pool(name="work", bufs=2))
        tri = ctx.enter_context(tc.tile_pool(name="tri", bufs=3))
        pacc = ctx.enter_context(tc.tile_pool(name="pacc", bufs=6, space="PSUM"))
        pmz = ctx.enter_context(tc.tile_pool(name="pmz", bufs=2, space="PSUM"))
        dram = ctx.enter_context(tc.tile_pool(name="dram", bufs=1, space="DRAM"))

        # ---- constants -----------------------------------------------------
        ident = const.tile([P, P], bf16, tag="ident")
        make_identity(nc, ident[:])
        ident2 = const.tile([P, P], bf16, tag="ident2")
        nc.vector.tensor_scalar_mul(ident2[:], ident[:], 2.0)
        iota_sb = const.tile([P, IND_B, P], bf16, tag="iota")
        nc.sync.dma_start(iota_sb[:], iota_d[:].rearrange("p (b q) -> p b q", q=P))
        deg_sb = const.tile([P, NW], fp32, tag="deg")
        nc.sync.dma_start(deg_sb[:], deg_d[:])
        dinv = const.tile([P, NW], fp32, tag="dinv")
        nc.scalar.sqrt(deg_sb[:], deg_sb[:])
        nc.vector.reciprocal(dinv[:], deg_sb[:])

        w0_sb = const.tile([P, KIN0, NH], bf16, tag="w0")
        nc.sync.dma_start(w0_sb[:], w0_d[:].rearrange("k p o -> p k o"))
        w12_sb = const.tile([P, 2, 2, NH], bf16, tag="w12")
        nc.sync.dma_start(w12_sb[:], w12_d[:].rearrange("l k p o -> p l k o"))
        wm0_sb = const.tile([P, 2, 2 * NH], bf16, tag="wm0")
        nc.sync.dma_start(wm0_sb[:], wm0_d[:].rearrange("k p o -> p k o"))
        wm1_sb = const.tile([P, 4, 64], bf16, tag="wm1")
        nc.sync.dma_start(wm1_sb[:], wm1_d[:].rearrange("k p o -> p k o"))
        b012_sb = const.tile([P, 3, NH], fp32, tag="b012")
        nc.sync.dma_start(b012_sb[:], b012_d[:].rearrange("l p o -> p l o"))
        bm0_sb = const.tile([P, 2 * NH], fp32, tag="bm0")
        nc.sync.dma_start(bm0_sb[:], bm0_d[:])
        bm1_sb = const.tile([P, 64], fp32, tag="bm1")
        nc.sync.dma_start(bm1_sb[:], bm1_d[:])

        # persistent transposed activations, 2 feature chunks of 128
        hTa = ht.tile([P, NW * P], bf16, tag="hTa")
        hTb = ht.tile([P, NW * P], bf16, tag="hTb")

        gsems = [nc.alloc_semaphore(f"gsem{q}") for q in range(4)]

        # ---- 3 GCN layers --------------------------------------------------
        for layer in range(DBG_LAYERS):
            ag_in = dram.tile([NW, P, NH], bf16, tag=f"agin{layer}")
            table = dram.tile([cfg.TBL_ROWS, NH], bf16,
                              tag=f"tbl{layer}", addr_space="Shared")

            # phase M: z = h @ W ; g = bf16(z * dinv) -> ag_in
            nkin = KIN0 if layer == 0 else 2
            g_stage = None
            for t in range(NW):
                if layer == 0:
                    h0t = tri.tile([P, KIN0 * P], bf16, tag="misc1",
                                   padded_shape=None)
                    nc.sync.dma_start(h0t[:], hT0_d[t])
                psum_z = pacc.tile([P, NH], fp32, tag="acc", name="psum_z")
                for kc in range(nkin):
                    if layer == 0:
                        lhsT = h0t[:, kc * P:(kc + 1) * P]
                        rhs = w0_sb[:, kc, :]
                    else:
                        lhsT = (hTa if kc == 0 else hTb)[:, t * P:(t + 1) * P]
                        rhs = w12_sb[:, layer - 1, kc, :]
                    nc.tensor.matmul(psum_z[:], lhsT, rhs,
                                     start=(kc == 0), stop=(kc == nkin - 1))
                if t % 8 == 0:
                    g_stage = tri.tile([P, 8, NH], bf16, tag="stage")
                nc.vector.tensor_scalar_mul(
                    g_stage[:, t % 8, :], psum_z[:], dinv[:, t:t + 1])
                if t % 8 == 7 or t == NW - 1:
                    nb = t % 8 + 1
                    t0 = t - nb + 1
                    nc.sync.dma_start(
                        ag_in[t0:t0 + nb].rearrange("t p f -> p t f"),
                        g_stage[:, :nb, :])

            nc.gpsimd.collective_compute(
                "AllGather", mybir.AluOpType.bypass,
                ins=[ag_in[:].opt()], outs=[table[:].opt()],
                replica_groups=[list(range(cfg.NCORES))],
            )

            # phase A: gather + indicator matmul segment-sum
            if not DBG_PHASE_A:
                break

            def postlude(w, acc):
                # h = relu(acc * dinv + bias); transpose into hTa/hTb
                tmp = tri.tile([P, NH], fp32, tag="pl_tmp", name="pl_tmp")
                nc.vector.scalar_tensor_tensor(
                    out=tmp[:], in0=acc[:],
                    scalar=dinv[:, w:w + 1],
                    in1=b012_sb[:, layer, :],
                    op0=mybir.AluOpType.mult,
                    op1=mybir.AluOpType.add)
                hbf = tri.tile([P, NH], bf16, tag="pl_hbf", name="pl_hbf")
                nc.scalar.activation(
                    hbf[:], tmp[:], mybir.ActivationFunctionType.Relu)
                for half, dst_t in ((0, hTa), (1, hTb)):
                    ptp = pmz.tile([P, P], bf16, tag="mz", name="ptp")
                    nc.tensor.transpose(
                        ptp[:], hbf[:, half * P:(half + 1) * P], ident[:])
                    nc.vector.tensor_copy(
                        dst_t[:, w * P:(w + 1) * P], ptp[:])

            def start_window(w):
                # self-loop term: acc = 2 * g_own[w]  (local rows, no gather)
                acc = pacc.tile([P, NH], fp32, tag="acc", name="acc")
                gown = work.tile([P, NH], bf16, tag="gown", name="gown",
                                 bufs=3)
                nc.sync.dma_start(gown[:], ag_in[w])
                return acc, gown

            qi = 0
            for s in range(cfg.NSW):
                g0 = sched["sw_g_off"][s]
                gcnt = sched["sw_g_cnt"][s]
                idx_sb = work.tile([P, G_SW_MAX * 8], i16, tag="idx")
                dstv_sb = work.tile([P, G_SW_MAX], bf16, tag="dstv")
                if gcnt > 0:
                    nc.sync.dma_start(idx_sb[:, :gcnt * 8],
                                      idx_d[:, g0 * 8:(g0 + gcnt) * 8])
                    nc.sync.dma_start(dstv_sb[:, :gcnt],
                                      dstv_d[:, g0:g0 + gcnt])
                gath = work.tile([P, G_SW_MAX, NH], bf16, tag="gath")
                prepped = []
                for (c, ng, goff) in sched["sw_calls"][s]:
                    if ng == 0:
                        continue
                    q = qi % 4
                    qi += 1
                    nc.gpsimd.dma_gather(
                        out_ap=gath[:, goff:goff + ng, :],
                        in_ap=table[c * cfg.CHUNK:(c + 1) * cfg.CHUNK, :],
                        idxs_ap=idx_sb[:, goff * 8:(goff + ng) * 8],
                        num_idxs=ng * P,
                        num_idxs_reg=ng * P,
                        elem_size=NH,
                        single_packet=False,
                        queue_num=q,
                        prepare_only=True,
                        sem=gsems[q],
                    )
                    prepped.append(q)
                for q in prepped:
                    nc.gpsimd.trigger_dma(count=None, queue_num=q)
                # indicator builds + matmuls, in group order
                ind8 = None
                accs = {}
                for gl in range(gcnt if DBG_PA_IND else 0):
                    w, st, sp = groups[g0 + gl]
                    if gl % IND_B == 0:
                        nb = min(IND_B, gcnt - gl)
                        ind8 = tri.tile([P, IND_B, P], bf16, tag="ind8")
                        nc.vector.tensor_tensor(
                            out=ind8[:, :nb, :],
                            in0=iota_sb[:, :nb, :],
                            in1=dstv_sb[:, gl:gl + nb].to_broadcast(
                                [P, nb, P]),
                            op=mybir.AluOpType.is_equal)
                    if not DBG_PA_MM:
                        continue
                    if st:
                        acc, gown = start_window(w)
                        accs[w] = acc
                        nc.tensor.matmul(acc[:], ident2[:], gown[:],
                                         start=True, stop=False)
                    nc.tensor.matmul(
                        accs[w][:], ind8[:, gl % IND_B, :],
                        gath[:, gl, :],
                        start=False, stop=bool(sp))
                    if sp and DBG_PA_POST:
                        postlude(w, accs[w])
                # windows in this superwindow with no edge groups at all
                if DBG_PA_MM:
                    w_lo = s * cfg.SW
                    w_hi = min((s + 1) * cfg.SW, NW)
                    for w in range(w_lo, w_hi):
                        if w in accs:
                            continue
                        acc, gown = start_window(w)
                        nc.tensor.matmul(acc[:], ident2[:], gown[:],
                                         start=True, stop=True)
                        if DBG_PA_POST:
                            postlude(w, acc)

        # ---- MLP head + log_softmax (8-tile waves, batched softmax) --------
        WAVE = 8
        for t0w in range(0, NW if DBG_HEAD else 0, WAVE):
            nwv = min(WAVE, NW - t0w)
            mbs = []
            for j in range(nwv):
                t = t0w + j
                psum_m = pacc.tile([P, 2 * NH], fp32, tag="acc",
                                   name="psum_m")
                for kc in range(2):
                    lhsT = (hTa if kc == 0 else hTb)[:, t * P:(t + 1) * P]
                    nc.tensor.matmul(psum_m[:], lhsT, wm0_sb[:, kc, :],
                                     start=(kc == 0), stop=(kc == 1))
                z0 = tri.tile([P, 2 * NH], fp32, tag="z0")
                nc.vector.tensor_add(z0[:], psum_m[:], bm0_sb[:])
                # elu(z) = relu(z) + min(exp(z) - 1, 0)
                ex = tri.tile([P, 2 * NH], bf16, tag="ex")
                nc.scalar.activation(ex[:], z0[:],
                                     mybir.ActivationFunctionType.Exp)
                nc.vector.tensor_scalar(
                    out=ex[:], in0=ex[:], scalar1=1.0, scalar2=0.0,
                    op0=mybir.AluOpType.subtract, op1=mybir.AluOpType.min)
                mb = tri.tile([P, 2 * NH], bf16, tag="mb", name="mb",
                              bufs=WAVE + 2)
                nc.vector.tensor_relu(mb[:], z0[:])
                nc.vector.tensor_add(mb[:], mb[:], ex[:])
                mbs.append(mb)
            lg8 = tri.tile([P, WAVE, 64], fp32, tag="lg8")
            for j in range(nwv):
                mT = tri.tile([P, 4, P], bf16, tag="misc1", name="mT")
                for q in range(4):
                    ptp = pmz.tile([P, P], bf16, tag="mz", name="ptp")
                    nc.tensor.transpose(ptp[:], mbs[j][:, q * P:(q + 1) * P],
                                        ident[:])
                    nc.vector.tensor_copy(mT[:, q, :], ptp[:])
                psum_l = pacc.tile([P, 64], fp32, tag="acc", name="psum_l")
                for q in range(4):
                    nc.tensor.matmul(psum_l[:], mT[:, q, :], wm1_sb[:, q, :],
                                     start=(q == 0), stop=(q == 3))
                nc.vector.tensor_add(lg8[:, j, :], psum_l[:], bm1_sb[:])
            # batched log_softmax over the wave
            mx8 = tri.tile([P, WAVE, 1], fp32, tag="mx8")
            nc.vector.tensor_reduce(mx8[:, :nwv, :], lg8[:, :nwv, :],
                                    axis=mybir.AxisListType.X,
                                    op=mybir.AluOpType.max)
            nc.vector.tensor_tensor(
                out=lg8[:, :nwv, :], in0=lg8[:, :nwv, :],
                in1=mx8[:, :nwv, :].to_broadcast([P, nwv, 64]),
                op=mybir.AluOpType.subtract)
            ex8 = tri.tile([P, WAVE, 64], bf16, tag="ex8")
            nc.scalar.activation(ex8[:, :nwv, :], lg8[:, :nwv, :],
                                 mybir.ActivationFunctionType.Exp)
            se8 = tri.tile([P, WAVE, 1], fp32, tag="se8")
            nc.vector.tensor_reduce(se8[:, :nwv, :], ex8[:, :nwv, :],
                                    axis=mybir.AxisListType.X,
                                    op=mybir.AluOpType.add)
            ln8 = tri.tile([P, WAVE, 1], fp32, tag="ln8")
            nc.scalar.activation(ln8[:, :nwv, :], se8[:, :nwv, :],
                                 mybir.ActivationFunctionType.Ln)
            out_stage = tri.tile([P, WAVE, 64], fp32, tag="stage")
            nc.vector.tensor_tensor(
                out=out_stage[:, :nwv, :], in0=lg8[:, :nwv, :],
                in1=ln8[:, :nwv, :].to_broadcast([P, nwv, 64]),
                op=mybir.AluOpType.subtract)
            nc.sync.dma_start(
                out_d[t0w:t0w + nwv].rearrange("t p f -> p t f"),
                out_stage[:, :nwv, :])
        if not DBG_HEAD:
            zt = tri.tile([P, 64], fp32, tag="zt")
            nc.vector.memset(zt[:], 0.0)
            nc.sync.dma_start(out_d[0].rearrange("p f -> p f"), zt[:])

    nc.compile()
    return nc


# ----------------------------------------------------------------------------
# entry point
# ----------------------------------------------------------------------------

_NC_CACHE = {}
TRACE = False
TRACE_KW = {}
LAST_RESULT = None


def _prepare(cfg, inputs):
    x = np.asarray(inputs["x"], np.float32)
    y = np.asarray(inputs["y"])
    adj = np.asarray(inputs["adj"])
    idx_labeled = np.asarray(inputs["idx_labeled"])

    h0 = _build_feats(cfg, x, y, idx_labeled)
    sched, dev_idx, dev_dstv, dev_deg = _build_schedule(cfg, adj)

    W0 = _pack_w(np.asarray(inputs["W0"], np.float32), cfg.KIN0)
    W1 = _pack_w(np.asarray(inputs["W1"], np.float32), 2)
    W2 = _pack_w(np.asarray(inputs["W2"], np.float32), 2)
    w12 = np.ascontiguousarray(np.stack([W1, W2]))
    Wm0 = _pack_w(np.asarray(inputs["Wm0"], np.float32), 2)
    Wm1 = _pack_w(np.asarray(inputs["Wm1"], np.float32), 4)
    b012 = np.ascontiguousarray(np.stack(
        [_bcast(np.asarray(inputs[k], np.float32)) for k in ("b0", "b1", "b2")]))
    bm0 = _bcast(np.asarray(inputs["bm0"], np.float32))
    bm1 = _bcast(np.asarray(inputs["bm1"], np.float32))
    iota = np.ascontiguousarray(np.broadcast_to(
        np.tile(np.arange(cfg.P, dtype=np.float32), cfg.IND_B)[None, :],
        (cfg.P, cfg.IND_B * cfg.P)).astype(BF16))

    in_maps = []
    for d in range(cfg.NCORES):
        h0_dev = np.zeros((cfg.NOWN_PAD, cfg.DIN), np.float32)
        h0_dev[:cfg.NOWN] = h0[d * cfg.NOWN:(d + 1) * cfg.NOWN]
        in_maps.append(dict(
            hT0=_pack_h0(cfg, h0_dev),
            idx=dev_idx[d], dstv=dev_dstv[d], deg=dev_deg[d],
            w0=W0, w12=w12, wm0=Wm0, wm1=Wm1,
            b012=b012, bm0=bm0, bm1=bm1, iota=iota,
        ))
    return sched, in_maps


def run(cfg, inputs):
    global LAST_RESULT
    from concourse.bass_utils import run_bass_kernel_spmd

    sched, in_maps = _prepare(cfg, inputs)
    key = (cfg, hashlib.sha1(
        np.asarray(inputs["adj"]).tobytes()).hexdigest())
    if key not in _NC_CACHE:
        _NC_CACHE[key] = _build_nc(cfg, sched)
    nc = _NC_CACHE[key]

    res = run_bass_kernel_spmd(
        nc, in_maps, core_ids=list(range(cfg.NCORES)),
        trace=TRACE, **TRACE_KW)
    LAST_RESULT = res
    outs = []
    for d in range(cfg.NCORES):
        o = res.results[d]["out"].reshape(cfg.NOWN_PAD, 64)
        outs.append(o[:cfg.NOWN])
    return np.ascontiguousarray(np.concatenate(outs, axis=0))


def kernel(**inputs) -> np.ndarray:
    return run(FULL, inputs)



# revision 3
# speedup vs baseline: 1.3091x; 1.3091x over previous
"""Trainium2 Bass kernel for nn_CLGNN_Model (3-layer GCN + MLP head + log_softmax).

Sharding: nodes are partitioned across 8 NeuronCores (12500 each).  Per GCN
layer, each core computes z = h @ W for its own nodes, scales rows by
dinv = rsqrt(deg), casts to fp8e4 and AllGathers the resulting "message table"
[100352, 256].  Edges are assigned to the core owning their destination;
the aggregation  acc[dst] = sum_{e->dst} g[src_e]  is computed with
dma_gather (int16-indexed row gather from the table, chunked into 4 source
ranges of 25088 rows to fit int16) followed by 0/1-indicator matmuls on the
TensorEngine that segment-sum 256 gathered edge rows at a time (fp8
DoubleRow pairs two 128-row groups per matmul) into a PSUM accumulator per
128-destination window.  Indicator matrices are precomputed on the host and
DMAed in, freeing the Vector engine.  Self-loops (weight 2.0) are a
2*I matmul on the core's own rows.  The instruction stream is identical
across cores (group counts are maxed over cores, short cells padded with
sentinel edges) so one SPMD program serves all 8 cores; only the data
arrays differ.
"""
import sys
import os
import hashlib
from dataclasses import dataclass

sys.path.insert(0, "/opt/trn_rl_repo")

import numpy as np
import ml_dtypes

BF16 = ml_dtypes.bfloat16
F8 = ml_dtypes.float8_e4m3

# ----------------------------------------------------------------------------
# configuration
# ----------------------------------------------------------------------------


@dataclass(frozen=True)
class Cfg:
    N: int = 100000           # total nodes
    NFEAT: int = 512
    NLABEL: int = 64
    NHID: int = 256
    NCORES: int = 8
    P: int = 128
    SW: int = 3               # windows per superwindow

    @property
    def NOWN(self):           # nodes per core
        return self.N // self.NCORES

    @property
    def NW(self):             # 128-windows per core
        return (self.NOWN + self.P - 1) // self.P

    @property
    def NOWN_PAD(self):
        return self.NW * self.P

    @property
    def TBL_ROWS(self):
        return self.NCORES * self.NOWN_PAD

    @property
    def CHUNK(self):          # table rows per int16-addressable chunk
        return 2 * self.NOWN_PAD

    @property
    def NCHUNK(self):
        return 4

    @property
    def NSW(self):
        return (self.NW + self.SW - 1) // self.SW

    @property
    def DIN(self):            # GCN layer-0 input dim
        return self.NFEAT + self.NLABEL

    @property
    def KIN0(self):           # 128-chunks of DIN (padded)
        return (self.DIN + self.P - 1) // self.P


FULL = Cfg()
PAD_DR = 200                 # dst_rel sentinel for padded edge slots

# ----------------------------------------------------------------------------
# host-side preprocessing
# ----------------------------------------------------------------------------


def _build_feats(cfg, x, y, idx_labeled):
    n = x.shape[0]
    idx = np.full((n,), cfg.NLABEL + 2, np.int64)
    idx[idx_labeled] = y[idx_labeled]
    feats = np.zeros((n, cfg.NLABEL), np.float32)
    lab = idx < cfg.NLABEL
    feats[np.nonzero(lab)[0], idx[lab]] = 1.0
    return np.concatenate([x, feats], axis=1)


def _build_schedule(cfg, adj):
    """Device-independent schedule + per-device index/indicator arrays."""
    P = cfg.P
    src = adj[0].astype(np.int64)
    dst = adj[1].astype(np.int64)

    indeg = np.bincount(dst, minlength=cfg.N).astype(np.float32)
    deg_full = indeg + 2.0

    # cells in schedule order: sw asc -> chunk asc -> window asc
    cells = []            # (sw, c, w)
    for s in range(cfg.NSW):
        ws = range(s * cfg.SW, min((s + 1) * cfg.SW, cfg.NW))
        for c in range(cfg.NCHUNK):
            for w in ws:
                cells.append((s, c, w))
    ncells = len(cells)

    # per-device edge -> cell assignment
    dev_edges = []        # (cell, src_local, dst_rel) arrays per device
    counts = np.zeros((cfg.NCORES, ncells), np.int64)
    sw_sizes = [min((s + 1) * cfg.SW, cfg.NW) - s * cfg.SW
                for s in range(cfg.NSW)]
    sw_off_arr = np.cumsum([0] + [sz * cfg.NCHUNK for sz in sw_sizes])[:-1]
    sw_sz_arr = np.array(sw_sizes)
    for d in range(cfg.NCORES):
        mask = (dst // cfg.NOWN) == d
        es = src[mask]
        ed = dst[mask]
        dl = ed - d * cfg.NOWN
        w = dl // P
        dst_rel = (dl % P).astype(np.uint8)
        trow = (es // cfg.NOWN) * cfg.NOWN_PAD + (es % cfg.NOWN)
        c = trow // cfg.CHUNK
        src_local = (trow % cfg.CHUNK).astype(np.int64)
        sw_of = w // cfg.SW
        w_in_sw = w - sw_of * cfg.SW
        cidv = sw_off_arr[sw_of] + c * sw_sz_arr[sw_of] + w_in_sw
        np.add.at(counts[d], cidv, 1)
        dev_edges.append((cidv, src_local, dst_rel))

    G = (counts.max(axis=0) + P - 1) // P   # groups per cell (0 if empty)
    cap = G * P
    cell_off = np.concatenate([[0], np.cumsum(cap)])       # edge offsets
    G_off = np.concatenate([[0], np.cumsum(G)])            # group offsets
    G_total = int(G.sum())
    total = int(cap.sum())

    # ---- per-sw gather calls + pair stream -------------------------------
    sw_calls = []         # per sw: [(chunk, ng, goff_rel)]
    sw_g_off = []         # sw group base (global)
    sw_g_cnt = []
    sw_pairs = []         # per sw: [(j_rel, w, start, stop)]
    sw_pair_off = []      # pair base (global, into indicator tensor)
    pair_groups = []      # global: [(ga, gb)] absolute group id or -1
    ci = 0
    for s in range(cfg.NSW):
        ws = list(range(s * cfg.SW, min((s + 1) * cfg.SW, cfg.NW)))
        g0 = G_off[ci]
        calls = []
        ci_start = ci
        for c in range(cfg.NCHUNK):
            ng = 0
            goff = G_off[ci]
            for _ in ws:
                ng += int(G[ci])
                ci += 1
            calls.append((c, ng, int(goff - g0)))
        gcnt = int(G_off[ci] - g0)
        assert gcnt >= 2, f"superwindow {s} has <2 groups"
        # pair stream in cell order
        stream = []       # (j_rel, w, ga, gb)
        cj = ci_start
        for c in range(cfg.NCHUNK):
            for w in ws:
                off = int(G_off[cj] - g0)
                Gc = int(G[cj])
                for p in range(Gc // 2):
                    stream.append((off + 2 * p, w,
                                   int(G_off[cj]) + 2 * p,
                                   int(G_off[cj]) + 2 * p + 1))
                if Gc % 2:
                    gl = int(G_off[cj]) + Gc - 1
                    if Gc >= 3:
                        stream.append((off + Gc - 2, w, -1, gl))
                    elif off >= 1:
                        stream.append((off - 1, w, -1, gl))
                    else:
                        stream.append((0, w, gl, -1))
                cj += 1
        # start/stop flags per window
        first = {}
        last = {}
        for i, (j, w, ga, gb) in enumerate(stream):
            if w not in first:
                first[w] = i
            last[w] = i
        sw_pairs.append([(j, w, int(i == first[w]), int(i == last[w]))
                         for i, (j, w, ga, gb) in enumerate(stream)])
        sw_pair_off.append(len(pair_groups))
        pair_groups.extend((ga, gb) for (j, w, ga, gb) in stream)
        sw_calls.append(calls)
        sw_g_off.append(int(g0))
        sw_g_cnt.append(gcnt)
    M_total = len(pair_groups)
    PAIRS_MAX = max(len(p) for p in sw_pairs)
    G_SW_MAX = max(sw_g_cnt)
    pair_groups = np.array(pair_groups, np.int64)          # [M, 2]

    # per-device data arrays
    dev_idx = []
    dev_ind = []
    dev_deg = []
    onehot = np.zeros((256, P), F8)
    onehot[np.arange(P), np.arange(P)] = 1.0
    for d in range(cfg.NCORES):
        cidv, src_local, dst_rel = dev_edges[d]
        # ascending src within each cell -> better HBM locality for gathers
        order = np.lexsort((src_local, cidv))
        cid_s = cidv[order]
        starts = np.searchsorted(cid_s, np.arange(ncells))
        within = np.arange(len(cid_s)) - starts[cid_s]
        pos = cell_off[cid_s] + within
        idx_flat = np.zeros(total, np.int64)
        dr_flat = np.full(total, PAD_DR, np.uint8)
        idx_flat[pos] = src_local[order]
        dr_flat[pos] = dst_rel[order]
        # wrapped int16 layout [128, total//16]
        a = idx_flat.reshape(total // 16, 16).T.astype(np.int16)
        dev_idx.append(np.ascontiguousarray(np.tile(a, (8, 1))))
        # indicators: [P(edge), M, 2, P(dst)] fp8
        dr_groups = dr_flat.reshape(G_total, P)
        slot_dr = np.full((M_total, 2, P), PAD_DR, np.uint8)
        for k in range(2):
            gk = pair_groups[:, k]
            valid = gk >= 0
            slot_dr[valid, k, :] = dr_groups[gk[valid]]
        ind = onehot[slot_dr]                   # [M, 2, Pedge, Pdst]
        dev_ind.append(np.ascontiguousarray(ind.transpose(2, 0, 1, 3)))
        dg = np.full((cfg.NOWN_PAD,), 1.0, np.float32)
        dg[:cfg.NOWN] = deg_full[d * cfg.NOWN:(d + 1) * cfg.NOWN]
        dev_deg.append(np.ascontiguousarray(
            dg.reshape(cfg.NW, P).T))          # [128, NW]

    sched = dict(
        sw_calls=sw_calls, sw_g_off=sw_g_off, sw_g_cnt=sw_g_cnt,
        sw_pairs=sw_pairs, sw_pair_off=sw_pair_off,
        G_SW_MAX=int(G_SW_MAX), PAIRS_MAX=int(PAIRS_MAX),
        G_total=G_total, M_total=M_total, S_total=G_total * 8,
    )
    return sched, dev_idx, dev_ind, dev_deg


def _pack_h0(cfg, h0_dev):
    """[NOWN_PAD, DIN] f32 -> [NW, 128, KIN0*128] bf16 lhsT-packed."""
    dpad = cfg.KIN0 * cfg.P
    h = np.zeros((cfg.NOWN_PAD, dpad), np.float32)
    h[:, :cfg.DIN] = h0_dev
    v = h.reshape(cfg.NW, cfg.P, cfg.KIN0, cfg.P)      # t, nc, kc, p
    return np.ascontiguousarray(v.transpose(0, 3, 2, 1)
                                .reshape(cfg.NW, cfg.P, cfg.KIN0 * cfg.P)
                                .astype(BF16))


def _pack_w(W, kin_chunks, p=128):
    """[K, O] -> [kin_chunks, 128, O] bf16 (zero-padded)."""
    K, O = W.shape
    Wp = np.zeros((kin_chunks * p, O), np.float32)
    Wp[:K] = W
    return np.ascontiguousarray(
        Wp.reshape(kin_chunks, p, O).astype(BF16))


def _bcast(b, p=128):
    return np.ascontiguousarray(np.broadcast_to(
        b.astype(np.float32)[None, :], (p, len(b))).copy())


# ----------------------------------------------------------------------------
# Bass program
# ----------------------------------------------------------------------------


def _build_nc(cfg, sched):
    from concourse import bass, mybir, tile, bacc
    from concourse.masks import make_identity
    from contextlib import ExitStack

    fp32 = mybir.dt.float32
    bf16 = mybir.dt.bfloat16
    fp8 = mybir.dt.float8e4
    i16 = mybir.dt.int16
    DR = mybir.MatmulPerfMode.DoubleRow
    P = cfg.P
    NH = cfg.NHID
    NW = cfg.NW
    KIN0 = cfg.KIN0
    G_SW_MAX = sched["G_SW_MAX"]
    PAIRS_MAX = sched["PAIRS_MAX"]

    nc = bacc.Bacc("TRN2", debug=False, num_swdge_queues=4)

    hT0_d = nc.dram_tensor("hT0", [NW, P, KIN0 * P], bf16, kind="ExternalInput")
    idx_d = nc.dram_tensor("idx", [P, sched["S_total"]], i16, kind="ExternalInput")
    ind_d = nc.dram_tensor("ind", [P, sched["M_total"], 2, P], fp8,
                           kind="ExternalInput")
    deg_d = nc.dram_tensor("deg", [P, NW], fp32, kind="ExternalInput")
    i2_d = nc.dram_tensor("i2", [P, P], fp8, kind="ExternalInput")
    w0_d = nc.dram_tensor("w0", [KIN0, P, NH], bf16, kind="ExternalInput")
    w12_d = nc.dram_tensor("w12", [2, 2, P, NH], bf16, kind="ExternalInput")
    wm0_d = nc.dram_tensor("wm0", [2, P, 2 * NH], bf16, kind="ExternalInput")
    wm1_d = nc.dram_tensor("wm1", [4, P, 64], bf16, kind="ExternalInput")
    b012_d = nc.dram_tensor("b012", [3, P, NH], fp32, kind="ExternalInput")
    bm0_d = nc.dram_tensor("bm0", [P, 2 * NH], fp32, kind="ExternalInput")
    bm1_d = nc.dram_tensor("bm1", [P, 64], fp32, kind="ExternalInput")
    out_d = nc.dram_tensor("out", [NW, P, 64], fp32, kind="ExternalOutput")

    with tile.TileContext(nc) as tc, ExitStack() as ctx:
        const = ctx.enter_context(tc.tile_pool(name="const", bufs=1))
        ht = ctx.enter_context(tc.tile_pool(name="ht", bufs=1))
        work = ctx.enter_context(tc.tile_pool(name="work", bufs=3))
        tri = ctx.enter_context(tc.tile_pool(name="tri", bufs=3))
        pacc = ctx.enter_context(tc.tile_pool(name="pacc", bufs=6, space="PSUM"))
        pmz = ctx.enter_context(tc.tile_pool(name="pmz", bufs=2, space="PSUM"))
        dram = ctx.enter_context(tc.tile_pool(name="dram", bufs=1, space="DRAM"))

        # ---- constants -----------------------------------------------------
        ident = const.tile([P, P], bf16, tag="ident")
        make_identity(nc, ident[:])
        ident2 = const.tile([P, P], fp8, tag="ident2")
        nc.sync.dma_start(ident2[:], i2_d[:])
        deg_sb = const.tile([P, NW], fp32, tag="deg")
        nc.sync.dma_start(deg_sb[:], deg_d[:])
        dinv = const.tile([P, NW], fp32, tag="dinv")
        nc.scalar.sqrt(deg_sb[:], deg_sb[:])
        nc.vector.reciprocal(dinv[:], deg_sb[:])

        w0_sb = const.tile([P, KIN0, NH], bf16, tag="w0")
        nc.sync.dma_start(w0_sb[:], w0_d[:].rearrange("k p o -> p k o"))
        w12_sb = const.tile([P, 2, 2, NH], bf16, tag="w12")
        nc.sync.dma_start(w12_sb[:], w12_d[:].rearrange("l k p o -> p l k o"))
        wm0_sb = const.tile([P, 2, 2 * NH], bf16, tag="wm0")
        nc.sync.dma_start(wm0_sb[:], wm0_d[:].rearrange("k p o -> p k o"))
        wm1_sb = const.tile([P, 4, 64], bf16, tag="wm1")
        nc.sync.dma_start(wm1_sb[:], wm1_d[:].rearrange("k p o -> p k o"))
        b012_sb = const.tile([P, 3, NH], fp32, tag="b012")
        nc.sync.dma_start(b012_sb[:], b012_d[:].rearrange("l p o -> p l o"))
        bm0_sb = const.tile([P, 2 * NH], fp32, tag="bm0")
        nc.sync.dma_start(bm0_sb[:], bm0_d[:])
        bm1_sb = const.tile([P, 64], fp32, tag="bm1")
        nc.sync.dma_start(bm1_sb[:], bm1_d[:])

        # persistent transposed activations, 2 feature chunks of 128
        hTa = ht.tile([P, NW * P], bf16, tag="hTa")
        hTb = ht.tile([P, NW * P], bf16, tag="hTb")

        # ---- 3 GCN layers --------------------------------------------------
        for layer in range(3):
            ag_in = dram.tile([NW, P, NH], fp8, tag=f"agin{layer}")
            table = dram.tile([cfg.TBL_ROWS, NH], fp8,
                              tag=f"tbl{layer}", addr_space="Shared")

            # phase M: z = h @ W ; g = fp8(z * dinv) -> ag_in
            nkin = KIN0 if layer == 0 else 2
            SLAB = 4
            g_stage = None
            h0slab = None
            for t in range(NW):
                if layer == 0:
                    if t % SLAB == 0:
                        nsl = min(SLAB, NW - t)
                        h0slab = tri.tile([P, SLAB, KIN0 * P], bf16,
                                          tag="h0slab", bufs=2)
                        nc.sync.dma_start(
                            h0slab[:, :nsl, :],
                            hT0_d[t:t + nsl].rearrange("t p f -> p t f"))
                psum_z = pacc.tile([P, NH], fp32, tag="acc", name="psum_z")
                for kc in range(nkin):
                    if layer == 0:
                        lhsT = h0slab[:, t % SLAB, kc * P:(kc + 1) * P]
                        rhs = w0_sb[:, kc, :]
                    else:
                        lhsT = (hTa if kc == 0 else hTb)[:, t * P:(t + 1) * P]
                        rhs = w12_sb[:, layer - 1, kc, :]
                    nc.tensor.matmul(psum_z[:], lhsT, rhs,
                                     start=(kc == 0), stop=(kc == nkin - 1))
                if t % 8 == 0:
                    g_stage = tri.tile([P, 8, NH], fp8, tag="stage")
                nc.vector.tensor_scalar_mul(
                    g_stage[:, t % 8, :], psum_z[:], dinv[:, t:t + 1])
                if t % 8 == 7 or t == NW - 1:
                    nb = t % 8 + 1
                    t0 = t - nb + 1
                    nc.sync.dma_start(
                        ag_in[t0:t0 + nb].rearrange("t p f -> p t f"),
                        g_stage[:, :nb, :])

            nc.gpsimd.collective_compute(
                "AllGather", mybir.AluOpType.bypass,
                ins=[ag_in[:].opt()], outs=[table[:].opt()],
                replica_groups=[list(range(cfg.NCORES))],
            )

            # phase A: gather + paired indicator matmul segment-sum
            def postlude(w, acc):
                # h = relu(acc * dinv + bias); transpose into hTa/hTb
                tmp = tri.tile([P, NH], fp32, tag="pl_tmp", name="pl_tmp")
                nc.vector.scalar_tensor_tensor(
                    out=tmp[:], in0=acc[:],
                    scalar=dinv[:, w:w + 1],
                    in1=b012_sb[:, layer, :],
                    op0=mybir.AluOpType.mult,
                    op1=mybir.AluOpType.add)
                hbf = tri.tile([P, NH], bf16, tag="pl_hbf", name="pl_hbf")
                nc.scalar.activation(
                    hbf[:], tmp[:], mybir.ActivationFunctionType.Relu)
                for half, dst_t in ((0, hTa), (1, hTb)):
                    ptp = pmz.tile([P, P], bf16, tag="mz", name="ptp")
                    nc.tensor.transpose(
                        ptp[:], hbf[:, half * P:(half + 1) * P], ident[:])
                    nc.vector.tensor_copy(
                        dst_t[:, w * P:(w + 1) * P], ptp[:])

            for s in range(cfg.NSW):
                w_lo = s * cfg.SW
                w_hi = min((s + 1) * cfg.SW, NW)
                nwin = w_hi - w_lo
                g0 = sched["sw_g_off"][s]
                gcnt = sched["sw_g_cnt"][s]
                pairs = sched["sw_pairs"][s]
                p0 = sched["sw_pair_off"][s]

                idx_sb = work.tile([P, G_SW_MAX * 8], i16, tag="idx")
                nc.sync.dma_start(idx_sb[:, :gcnt * 8],
                                  idx_d[:, g0 * 8:(g0 + gcnt) * 8])
                ind_sb = work.tile([P, PAIRS_MAX, 2, P], fp8, tag="ind")
                nc.sync.dma_start(ind_sb[:, :len(pairs), :, :],
                                  ind_d[:, p0:p0 + len(pairs), :, :])
                gown = work.tile([P, cfg.SW, NH], fp8, tag="gown")
                nc.sync.dma_start(
                    gown[:, :nwin, :],
                    ag_in[w_lo:w_hi].rearrange("t p f -> p t f"))
                gath = work.tile([P, G_SW_MAX + 1, NH], fp8, tag="gath")
                for (c, ng, goff) in sched["sw_calls"][s]:
                    if ng == 0:
                        continue
                    nc.gpsimd.dma_gather(
                        out_ap=gath[:, goff:goff + ng, :],
                        in_ap=table[c * cfg.CHUNK:(c + 1) * cfg.CHUNK, :],
                        idxs_ap=idx_sb[:, goff * 8:(goff + ng) * 8],
                        num_idxs=ng * P,
                        num_idxs_reg=ng * P,
                        elem_size=NH,
                        single_packet=False,
                        queue_num=c,
                    )
                accs = {}
                for pl, (j, w, st, sp) in enumerate(pairs):
                    if st:
                        acc = pacc.tile([P, NH], fp32, tag="acc", name="acc")
                        accs[w] = acc
                        nc.tensor.matmul(acc[:], ident2[:],
                                         gown[:, w - w_lo, :],
                                         start=True, stop=False)
                    nc.tensor.matmul(
                        accs[w][:], ind_sb[:, pl, :, :],
                        gath[:, j:j + 2, :],
                        start=False, stop=bool(sp), perf_mode=DR)
                    if sp:
                        postlude(w, accs[w])
                # windows in this superwindow with no edge groups at all
                for w in range(w_lo, w_hi):
                    if w in accs:
                        continue
                    acc = pacc.tile([P, NH], fp32, tag="acc", name="acc")
                    nc.tensor.matmul(acc[:], ident2[:], gown[:, w - w_lo, :],
                                     start=True, stop=True)
                    postlude(w, acc)

        # ---- MLP head + log_softmax (8-tile waves, batched softmax) --------
        WAVE = 8
        for t0w in range(0, NW, WAVE):
            nwv = min(WAVE, NW - t0w)
            mbs = []
            for j in range(nwv):
                t = t0w + j
                psum_m = pacc.tile([P, 2 * NH], fp32, tag="acc",
                                   name="psum_m")
                for kc in range(2):
                    lhsT = (hTa if kc == 0 else hTb)[:, t * P:(t + 1) * P]
                    nc.tensor.matmul(psum_m[:], lhsT, wm0_sb[:, kc, :],
                                     start=(kc == 0), stop=(kc == 1))
                z0 = tri.tile([P, 2 * NH], fp32, tag="z0")
                nc.vector.tensor_add(z0[:], psum_m[:], bm0_sb[:])
                # elu(z) = relu(z) + min(exp(z) - 1, 0)
                ex = tri.tile([P, 2 * NH], bf16, tag="ex")
                nc.scalar.activation(ex[:], z0[:],
                                     mybir.ActivationFunctionType.Exp)
                nc.vector.tensor_scalar(
                    out=ex[:], in0=ex[:], scalar1=1.0, scalar2=0.0,
                    op0=mybir.AluOpType.subtract, op1=mybir.AluOpType.min)
                mb = tri.tile([P, 2 * NH], bf16, tag="mb", name="mb",
                              bufs=WAVE + 2)
                nc.vector.tensor_relu(mb[:], z0[:])
                nc.vector.tensor_add(mb[:], mb[:], ex[:])
                mbs.append(mb)
            lg8 = tri.tile([P, WAVE, 64], fp32, tag="lg8")
            for j in range(nwv):
                mT = tri.tile([P, 4, P], bf16, tag="misc1", name="mT")
                for q in range(4):
                    ptp = pmz.tile([P, P], bf16, tag="mz", name="ptp")
                    nc.tensor.transpose(ptp[:], mbs[j][:, q * P:(q + 1) * P],
                                        ident[:])
                    nc.vector.tensor_copy(mT[:, q, :], ptp[:])
                psum_l = pacc.tile([P, 64], fp32, tag="acc", name="psum_l")
                for q in range(4):
                    nc.tensor.matmul(psum_l[:], mT[:, q, :], wm1_sb[:, q, :],
                                     start=(q == 0), stop=(q == 3))
                nc.vector.tensor_add(lg8[:, j, :], psum_l[:], bm1_sb[:])
            # batched log_softmax over the wave
            mx8 = tri.tile([P, WAVE, 1], fp32, tag="mx8")
            nc.vector.tensor_reduce(mx8[:, :nwv, :], lg8[:, :nwv, :],
                                    axis=mybir.AxisListType.X,
                                    op=mybir.AluOpType.max)
            nc.vector.tensor_tensor(
                out=lg8[:, :nwv, :], in0=lg8[:, :nwv, :],
                in1=mx8[:, :nwv, :].to_broadcast([P, nwv, 64]),
                op=mybir.AluOpType.subtract)
            ex8 = tri.tile([P, WAVE, 64], bf16, tag="ex8")
            nc.scalar.activation(ex8[:, :nwv, :], lg8[:, :nwv, :],
                                 mybir.ActivationFunctionType.Exp)
            se8 = tri.tile([P, WAVE, 1], fp32, tag="se8")
            nc.vector.tensor_reduce(se8[:, :nwv, :], ex8[:, :nwv, :],
                                    axis=mybir.AxisListType.X,
                                    op=mybir.AluOpType.add)
            ln8 = tri.tile([P, WAVE, 1], fp32, tag="ln8")
            nc.scalar.activation(ln8[:, :nwv, :], se8[:, :nwv, :],
                                 mybir.ActivationFunctionType.Ln)
            out_stage = tri.tile([P, WAVE, 64], fp32, tag="ostage")
            nc.vector.tensor_tensor(
                out=out_stage[:, :nwv, :], in0=lg8[:, :nwv, :],
                in1=ln8[:, :nwv, :].to_broadcast([P, nwv, 64]),
                op=mybir.AluOpType.subtract)
            nc.sync.dma_start(
                out_d[t0w:t0w + nwv].rearrange("t p f -> p t f"),
                out_stage[:, :nwv, :])

    nc.compile()
    return nc


# ----------------------------------------------------------------------------
# entry point
# ----------------------------------------------------------------------------

_NC_CACHE = {}
TRACE = False
TRACE_KW = {}
LAST_RESULT = None


def _prepare(cfg, inputs):
    x = np.asarray(inputs["x"], np.float32)
    y = np.asarray(inputs["y"])
    adj = np.asarray(inputs["adj"])
    idx_labeled = np.asarray(inputs["idx_labeled"])

    h0 = _build_feats(cfg, x, y, idx_labeled)
    sched, dev_idx, dev_ind, dev_deg = _build_schedule(cfg, adj)

    W0 = _pack_w(np.asarray(inputs["W0"], np.float32), cfg.KIN0)
    W1 = _pack_w(np.asarray(inputs["W1"], np.float32), 2)
    W2 = _pack_w(np.asarray(inputs["W2"], np.float32), 2)
    w12 = np.ascontiguousarray(np.stack([W1, W2]))
    Wm0 = _pack_w(np.asarray(inputs["Wm0"], np.float32), 2)
    Wm1 = _pack_w(np.asarray(inputs["Wm1"], np.float32), 4)
    b012 = np.ascontiguousarray(np.stack(
        [_bcast(np.asarray(inputs[k], np.float32)) for k in ("b0", "b1", "b2")]))
    bm0 = _bcast(np.asarray(inputs["bm0"], np.float32))
    bm1 = _bcast(np.asarray(inputs["bm1"], np.float32))
    i2 = np.ascontiguousarray((2.0 * np.eye(cfg.P, dtype=np.float32))
                              .astype(F8))

    in_maps = []
    for d in range(cfg.NCORES):
        h0_dev = np.zeros((cfg.NOWN_PAD, cfg.DIN), np.float32)
        h0_dev[:cfg.NOWN] = h0[d * cfg.NOWN:(d + 1) * cfg.NOWN]
        in_maps.append(dict(
            hT0=_pack_h0(cfg, h0_dev),
            idx=dev_idx[d], ind=dev_ind[d], deg=dev_deg[d], i2=i2,
            w0=W0, w12=w12, wm0=Wm0, wm1=Wm1,
            b012=b012, bm0=bm0, bm1=bm1,
        ))
    return sched, in_maps


def run(cfg, inputs):
    global LAST_RESULT
    from concourse.bass_utils import run_bass_kernel_spmd

    sched, in_maps = _prepare(cfg, inputs)
    key = (cfg, hashlib.sha1(
        np.asarray(inputs["adj"]).tobytes()).hexdigest())
    if key not in _NC_CACHE:
        _NC_CACHE[key] = _build_nc(cfg, sched)
    nc = _NC_CACHE[key]

    res = run_bass_kernel_spmd(
        nc, in_maps, core_ids=list(range(cfg.NCORES)),
        trace=TRACE, **TRACE_KW)
    LAST_RESULT = res
    outs = []
    for d in range(cfg.NCORES):
        o = res.results[d]["out"].reshape(cfg.NOWN_PAD, 64)
        outs.append(o[:cfg.NOWN])
    return np.ascontiguousarray(np.concatenate(outs, axis=0))


def kernel(**inputs) -> np.ndarray:
    return run(FULL, inputs)
